# revision 45
# baseline (speedup 1.0000x reference)
"""ArcFace logits on 8 Trainium2 NeuronCores — class-parallel (partial-FC) sharding.

Math: logits = SCALE * cos(arccos(clip(f_n @ w_n.T)) + MARGIN*onehot(targets))
Since cos(arccos(x)) == x, only the 1024 target entries need the margin
correction cos(t+m) = cos(m)*x - sin(m)*sqrt(1-x^2); everything else is just
the normalized matmul scaled by SCALE.

Device (SPMD, identical graph on all 8 cores, class-sharded):
  - f row-normalize (*SCALE folded in), cast bf16, PE-transpose -> fT
  - main matmul out[c,b] = wT.T @ fT in bf16 (fp32 PSUM), w-norm scaling fused
    into the PSUM->SBUF evacuation (per-partition tensor_scalar), fp16 out
  - w column norms: ACT Square -> k-presum (GPSIMD) -> one N=1 bf16
    ones-matmul per 128-class block -> DVE reciprocal + ACT Sqrt
  - margin deltas for all 1024 rows from gathered target weight rows
    (priority-demoted so the static scheduler slots it into mid-kernel slack)
DMA: all inputs + outputs on the two HWDGE rings (sync/scalar); weight chunks
prefetched two ahead; preamble loads (f, w0, w1) split across both rings.
Host: shard/transpose/concat + apply the device-computed deltas at the 1024
target positions (pure indexing; all arithmetic happens on device).
"""

import math
import os

import numpy as np

IN_F = 512
OUT_C = 100000
B = 1024
MARGIN = 0.5
SCALE = 20.0

NCORES = 8
CSH = 12800            # classes per core after padding
CPAD = CSH * NCORES    # 102400
P = 128
KT = IN_F // P         # 4 contraction subtiles
BT = B // P            # 8 batch tiles
NF = 512               # matmul moving free dim (one PSUM bank of fp32)
NB = B // NF           # 2
CW = 1280              # class chunk width streamed from DRAM
CBK = CW // P          # 10 class blocks per chunk
CHUNKS = CSH // CW     # 10
OG = 5                 # c-blocks batched per output DMA

_GRAPH = None
LAST_EXEC_TIME_NS = None


def _build_graph():
    from contextlib import ExitStack

    import concourse.bass as bass  # noqa: F401
    import concourse.tile as tile
    from concourse import bacc, mybir

    dt = mybir.dt
    AF = mybir.ActivationFunctionType
    ALU = mybir.AluOpType
    cosm = math.cos(MARGIN)
    sinm = math.sin(MARGIN)

    nc = bacc.Bacc()
    wT_e = nc.declare_dram_parameter("wT", [IN_F, CSH], dt.bfloat16, isOutput=False)
    fT_e = nc.declare_dram_parameter("fT", [IN_F, B], dt.bfloat16, isOutput=False)
    f_e = nc.declare_dram_parameter("f", [B, IN_F], dt.bfloat16, isOutput=False)
    wtg_e = nc.declare_dram_parameter("wtgt", [B, IN_F], dt.bfloat16, isOutput=False)
    out_e = nc.declare_dram_parameter("out", [CSH, B], dt.float16, isOutput=True)
    dlt_e = nc.declare_dram_parameter("delta", [P, BT], dt.float32, isOutput=True)

    wT_v = wT_e[:].rearrange("(k p) c -> p k c", p=P)   # d = k*128 + p
    fT_v = fT_e[:].rearrange("(k p) b -> p k b", p=P)   # d = k*128 + p
    f_v = f_e[:].rearrange("(t p) d -> p t d", p=P)     # b = t*128 + p
    wtg_v = wtg_e[:].rearrange("(t p) d -> p t d", p=P)

    with ExitStack() as ctx:
        tc = ctx.enter_context(tile.TileContext(nc))
        cpool = ctx.enter_context(tc.tile_pool(name="cpool", bufs=1))
        fpool = ctx.enter_context(tc.tile_pool(name="fpool", bufs=1))
        wpool = ctx.enter_context(tc.tile_pool(name="wpool", bufs=3))
        sqpool = ctx.enter_context(tc.tile_pool(name="sqpool", bufs=2))
        opool = ctx.enter_context(tc.tile_pool(name="opool", bufs=3))
        smal = ctx.enter_context(tc.tile_pool(name="smal", bufs=2))
        pn_pool = ctx.enter_context(tc.tile_pool(name="pn", bufs=1, space="PSUM"))
        po_pool = ctx.enter_context(tc.tile_pool(name="po", bufs=7, space="PSUM"))

        # ---------------- input DMAs first; fT / w0 / w1 split across both
        # HWDGE rings so the preamble pipeline fills as fast as possible ----
        fT_raw = fpool.tile([P, KT, B], dt.bfloat16)
        nc.sync.dma_start(fT_raw[:, 0 : KT // 2], fT_v[:, 0 : KT // 2])
        nc.scalar.dma_start(fT_raw[:, KT // 2 :], fT_v[:, KT // 2 :])

        def emit_load(ci, split=False):
            w_sb = wpool.tile([P, KT, CW], dt.bfloat16, tag="wchunk", name="w_sb")
            src = wT_v[:, :, ci * CW : (ci + 1) * CW]
            if split:
                nc.sync.dma_start(w_sb[:, 0 : KT // 2], src[:, 0 : KT // 2])
                nc.scalar.dma_start(w_sb[:, KT // 2 :], src[:, KT // 2 :])
            else:
                eng = nc.sync if ci % 2 == 0 else nc.scalar
                eng.dma_start(w_sb[:], src)
            return w_sb

        w_sbs = {0: emit_load(0, split=True), 1: emit_load(1, split=True)}

        # margin-path inputs (needed only mid-kernel) queue behind the above
        wt_sb = fpool.tile([P, BT, IN_F], dt.bfloat16, name="wt_sb")
        nc.sync.dma_start(wt_sb[:], wtg_v)
        f_sb = fpool.tile([P, BT, IN_F], dt.bfloat16)
        nc.scalar.dma_start(f_sb[:], f_v)

        ones_b = cpool.tile([P, 1], dt.bfloat16)
        nc.gpsimd.memset(ones_b[:], 1.0)
        ones_r = cpool.tile([1, P], dt.bfloat16)
        nc.gpsimd.memset(ones_r[:], 1.0)
        # dummy Sqrt so the one-time ACT_TABLE_LOAD (~1.3us) happens during
        # the input DMA wait instead of on the rnf critical chain
        warm = cpool.tile([1, 1], dt.float32)
        nc.scalar.activation(warm[:], ones_b[0:1, 0:1], AF.Sqrt, scale=1.0)

        # ---------------- f path: batch norms without any transposes -------
        # |f_b|^2 via Square + k-presum + ones-stationary row-matmul, then
        # SCALE*rsqrt broadcast to all partitions with a K=1 outer-product
        # matmul; fT is normalized in place of the old mul+transpose chain.
        fT_sq = sqpool.tile([P, KT, B], dt.bfloat16, tag="ftsq", name="ftsq")
        nc.scalar.activation(fT_sq[:], fT_raw[:], AF.Square)
        fsum = sqpool.tile([P, B], dt.bfloat16, tag="fsum", name="fsum")
        nc.vector.tensor_add(fsum[:], fT_sq[:, 0], fT_sq[:, 1])
        nc.vector.tensor_add(fsum[:], fsum[:], fT_sq[:, 2])
        nc.vector.tensor_add(fsum[:], fsum[:], fT_sq[:, 3])

        rnf_rec = smal.tile([1, B], dt.float32, tag="rnfrec")
        rnf_row = smal.tile([1, B], dt.bfloat16, tag="rnfrow")
        rnf_bc = fpool.tile([P, B], dt.bfloat16, name="rnfbc")
        psrs = []
        for h in range(NB):
            psr = po_pool.tile([P, NF], dt.float32, tag="pso", name="psr")
            nc.tensor.matmul(
                psr[0:1, :],
                lhsT=ones_b[:],
                rhs=fsum[:, h * NF : (h + 1) * NF],
                start=True,
                stop=True,
            )
            psrs.append(psr)
        for h in range(NB):
            # 1/nf2 at ~18 bits (values ~512, far from the undefined edges)
            nc.vector.reciprocal_approx_fast(
                rnf_rec[:, h * NF : (h + 1) * NF], psrs[h][0:1, :]
            )
        for h in range(NB):
            # sqrt(SCALE^2 / nf2) = SCALE * rsqrt(nf2)
            nc.scalar.activation(
                rnf_row[:, h * NF : (h + 1) * NF],
                rnf_rec[:, h * NF : (h + 1) * NF],
                AF.Sqrt,
                scale=SCALE * SCALE,
            )
        psbs = []
        for h in range(NB):
            psb = po_pool.tile([P, NF], dt.float32, tag="pso", name="psb")
            nc.tensor.matmul(
                psb[:],
                lhsT=ones_r[:],
                rhs=rnf_row[:, h * NF : (h + 1) * NF],
                start=True,
                stop=True,
            )
            psbs.append(psb)
        for h in range(NB):
            nc.vector.tensor_copy(rnf_bc[:, h * NF : (h + 1) * NF], psbs[h][:])

        fT = fpool.tile([P, KT, B], dt.bfloat16)
        for k in range(KT):
            nc.vector.tensor_mul(fT[:, k], fT_raw[:, k], rnf_bc[:])

        # ---------------- w-norm pipeline pieces --------------------------
        def emit_sqsum(ci):
            """ACT square of chunk ci, then DVE k-presum -> wsum bf16."""
            w_sb = w_sbs[ci]
            wsq = sqpool.tile([P, KT, CW], dt.bfloat16, tag="wsq", name="wsq")
            nc.scalar.activation(wsq[:], w_sb[:], AF.Square)
            wsum = sqpool.tile([P, CW], dt.bfloat16, tag="wsum", name="wsum")
            nc.vector.tensor_add(wsum[:], wsq[:, 0], wsq[:, 1])
            nc.vector.tensor_add(wsum[:], wsum[:], wsq[:, 2])
            nc.vector.tensor_add(wsum[:], wsum[:], wsq[:, 3])
            return wsum

        def emit_sqsum_part(w_sb, c0, c1):
            """Square + k-presum for a column slice of a chunk."""
            wsq = sqpool.tile([P, KT, c1 - c0], dt.bfloat16, tag="wsqp", name="wsqp")
            nc.scalar.activation(wsq[:], w_sb[:, :, c0:c1], AF.Square)
            wsum = sqpool.tile([P, c1 - c0], dt.bfloat16, tag="wsump", name="wsump")
            nc.vector.tensor_add(wsum[:], wsq[:, 0], wsq[:, 1])
            nc.vector.tensor_add(wsum[:], wsum[:], wsq[:, 2])
            nc.vector.tensor_add(wsum[:], wsum[:], wsq[:, 3])
            return wsum

        # chunk0's og0 norm slice (ready early so its evacuations can start
        # as soon as the first matmul banks stop)
        wsum0a = emit_sqsum_part(w_sbs[0], 0, OG * P)

        def emit_normmm(ci, wsum, nblk=CBK, tag=""):
            """Per-class norm^2 via one N=1 ones-matmul per 128-class block."""
            nps = pn_pool.tile([P, nblk], dt.float32, tag=f"nps{tag}", name="nps")
            for cb in range(nblk):
                nc.tensor.matmul(
                    nps[:, cb : cb + 1],
                    lhsT=wsum[:, cb * P : (cb + 1) * P],
                    rhs=ones_b[:],
                    start=True,
                    stop=True,
                )
            recw = smal.tile([P, nblk], dt.float32, tag=f"recw{tag}", name="recw")
            nc.vector.reciprocal(recw[:], nps[:])
            rnw = smal.tile([P, nblk], dt.float32, tag=f"rnw{tag}", name="rnw")
            nc.scalar.activation(rnw[:], recw[:], AF.Sqrt, scale=1.0)
            return rnw

        nt2 = smal.tile([P, BT], dt.float32)
        nf2m = smal.tile([P, BT], dt.float32)
        drot = smal.tile([P, BT], dt.float32)

        def emit_margin_part(ts):
            for t in ts:
                sq = sqpool.tile([P, IN_F], dt.float32, tag="sqscratch")
                nc.scalar.activation(
                    sq[:], wt_sb[:, t], AF.Square, accum_out=nt2[:, t : t + 1]
                )
                sqf = sqpool.tile([P, IN_F], dt.float32, tag="sqscratch")
                nc.scalar.activation(
                    sqf[:], f_sb[:, t], AF.Square, accum_out=nf2m[:, t : t + 1]
                )
                prod = sqpool.tile([P, IN_F], dt.float32, tag="prodscratch")
                nc.vector.tensor_mul(prod[:], f_sb[:, t], wt_sb[:, t])
                nc.vector.reduce_sum(
                    drot[:, t : t + 1], prod[:], axis=mybir.AxisListType.X
                )

        def emit_margin_tail():
            rec_t = smal.tile([P, BT], dt.float32)
            nc.vector.reciprocal(rec_t[:], nt2[:])
            rnt = smal.tile([P, BT], dt.float32)
            nc.scalar.activation(rnt[:], rec_t[:], AF.Sqrt, scale=1.0)
            rec_f = smal.tile([P, BT], dt.float32)
            nc.vector.reciprocal(rec_f[:], nf2m[:])
            rnf20 = smal.tile([P, BT], dt.float32)
            nc.scalar.activation(rnf20[:], rec_f[:], AF.Sqrt, scale=SCALE * SCALE)
            u = smal.tile([P, BT], dt.float32)
            nc.vector.tensor_mul(u[:], drot[:], rnf20[:])
            nc.vector.tensor_mul(u[:], u[:], rnt[:])          # u = SCALE * cos_t
            t1 = smal.tile([P, BT], dt.float32)
            nc.vector.tensor_mul(t1[:], u[:], u[:])
            nc.vector.tensor_scalar(t1[:], t1[:], -1.0, SCALE * SCALE, ALU.mult, ALU.add)
            nc.vector.tensor_scalar_max(t1[:], t1[:], 0.0)    # max(S^2 - u^2, 0)
            s_t = smal.tile([P, BT], dt.float32)
            nc.scalar.activation(s_t[:], t1[:], AF.Sqrt, scale=1.0)  # SCALE*sin_t
            t2 = smal.tile([P, BT], dt.float32)
            nc.vector.tensor_scalar_mul(t2[:], s_t[:], -sinm)
            t3 = smal.tile([P, BT], dt.float32)
            nc.vector.tensor_scalar_mul(t3[:], u[:], cosm - 1.0)
            delta = smal.tile([P, BT], dt.float32)
            nc.vector.tensor_add(delta[:], t2[:], t3[:])
            nc.gpsimd.dma_start(dlt_e[:], delta[:])

        # ---------------- main class loop ---------------------------------
        # chunk0's norm runs in the preamble (PE waits on it once);
        # chunk i+1's square/presum is emitted at og1 of chunk i and its
        # norm-matmuls at the end of chunk i, so steady-state PE never waits.
        rnw = emit_normmm(0, wsum0a, nblk=OG)
        rnw_off = 0

        for ci in range(CHUNKS):
            w_sb = w_sbs.pop(ci)
            for og in range(CBK // OG):
                if ci == 0 and og == 1:
                    # second half of chunk0's norm, between og0 and og1 MMs
                    rnw = emit_normmm(
                        0, emit_sqsum_part(w_sb, OG * P, CW), nblk=OG
                    )
                    rnw_off = OG
                if og == 0 and ci + 2 < CHUNKS:
                    w_sbs[ci + 2] = emit_load(ci + 2)
                osb = opool.tile([P, OG, NB * NF], dt.float16, tag="osb")
                for cbi in range(OG):
                    cb = og * OG + cbi
                    psos = [
                        po_pool.tile([P, NF], dt.float32, tag="pso", name=f"pso{nb}")
                        for nb in range(NB)
                    ]
                    for k in range(KT):
                        for nb in range(NB):
                            nc.tensor.matmul(
                                psos[nb][:],
                                lhsT=w_sb[:, k, cb * P : (cb + 1) * P],
                                rhs=fT[:, k, nb * NF : (nb + 1) * NF],
                                start=(k == 0),
                                stop=(k == KT - 1),
                            )
                    if og == 1 and cbi == 3 and ci + 1 < CHUNKS:
                        # norm-matmuls for the next chunk ride mid-og1 (wsum
                        # is ready by now), so rnw(ci+1) beats the chunk
                        # boundary and og0's evacuations never wait on it
                        new_rnw = emit_normmm(ci + 1, next_wsum)
                    ri = cb - rnw_off
                    for nb in range(NB):
                        if (cb + nb) % 2 == 0:
                            nc.scalar.activation(
                                osb[:, cbi, nb * NF : (nb + 1) * NF],
                                psos[nb][:], AF.Copy,
                                scale=rnw[:, ri : ri + 1],
                            )
                        else:
                            nc.vector.tensor_scalar_mul(
                                osb[:, cbi, nb * NF : (nb + 1) * NF],
                                psos[nb][:], rnw[:, ri : ri + 1],
                            )
                row0 = ci * CW + og * OG * P
                if ci == CHUNKS - 1 and og == 1:
                    # fine-grained split of the final burst across both
                    # rings, per block, so the drain tail is minimal
                    for g in range(OG):
                        eng = nc.sync if g % 2 == 0 else nc.scalar
                        eng.dma_start(
                            out_e[row0 + g * P : row0 + (g + 1) * P, :].rearrange(
                                "(g p) b -> p g b", p=P
                            ),
                            osb[:, g : g + 1],
                        )
                else:
                    dma_eng = nc.sync if og % 2 == 0 else nc.scalar
                    dma_eng.dma_start(
                        out_e[row0 : row0 + OG * P, :].rearrange(
                            "(g p) b -> p g b", p=P
                        ),
                        osb[:],
                    )
                if og == 0 and ci + 1 < CHUNKS:
                    # square + presum for the next chunk start at og0's end:
                    # ACT runs it after og0's evacuations, DVE after og0's,
                    # and wsum is ready before mid-og1
                    next_wsum = emit_sqsum(ci + 1)
            if ci + 1 < CHUNKS:
                rnw = new_rnw
                rnw_off = 0
            # margin work split into small slots across chunks so its
            # ACT/DVE bursts never starve the evacuations; tile_wait_until
            # pins each slot to mid-kernel model time so the static scheduler
            # cannot hoist it to the stream heads (it would head-of-line
            # block the f path while waiting on the wtgt/f DMAs)
            if 2 <= ci <= 5:
                with tc.tile_wait_until(0.055 + 0.02 * (ci - 2)):
                    emit_margin_part(range((ci - 2) * 2, (ci - 2) * 2 + 2))
            elif ci == 6:
                with tc.tile_wait_until(0.135):
                    emit_margin_tail()

    nc.finalize()
    return nc


def _prep_inputs(features, targets, weights):
    import ml_dtypes

    f32 = np.asarray(features, dtype=np.float32)
    fbf = np.ascontiguousarray(f32.astype(ml_dtypes.bfloat16))
    fTbf = np.ascontiguousarray(fbf.T)
    tgt = np.asarray(targets).astype(np.int64)
    w = np.asarray(weights, dtype=np.float32)

    wpad = np.zeros((CPAD, IN_F), dtype=np.float32)
    wpad[:OUT_C] = w
    wpad[OUT_C:, 0] = 1.0  # unit-norm filler rows: no inf/nan anywhere

    in_maps = []
    for i in range(NCORES):
        sh = wpad[i * CSH : (i + 1) * CSH]
        wT = np.ascontiguousarray(sh.astype(ml_dtypes.bfloat16).T)
        loc = np.clip(tgt - i * CSH, 0, CSH - 1)
        wtgt = np.ascontiguousarray(sh[loc].astype(ml_dtypes.bfloat16))
        in_maps.append({"wT": wT, "fT": fTbf, "f": fbf, "wtgt": wtgt})
    return in_maps, tgt


def kernel(features, targets, weights):
    global _GRAPH, LAST_EXEC_TIME_NS
    from concourse.bass_utils import run_bass_kernel_spmd

    if _GRAPH is None:
        _GRAPH = _build_graph()
    nc = _GRAPH

    in_maps, tgt = _prep_inputs(features, targets, weights)

    trace = bool(int(os.environ.get("BASS_KERNEL_TRACE", "0")))
    res = run_bass_kernel_spmd(nc, in_maps, core_ids=list(range(NCORES)), trace=trace)
    LAST_EXEC_TIME_NS = res.exec_time_ns

    outs = [res.results[i]["out"] for i in range(NCORES)]       # [CSH, B] f16
    full = np.concatenate(outs, axis=0)[:OUT_C]                 # [OUT_C, B]
    logits = np.ascontiguousarray(full.T, dtype=np.float32)     # [B, OUT_C]

    # apply device-computed margin deltas at the 1024 target positions
    deltas = np.stack(
        [res.results[i]["delta"].T.reshape(B) for i in range(NCORES)]
    )  # [NCORES, B]; delta[p, t] -> b = t*128 + p
    rows = np.arange(B)
    core_of = (tgt // CSH).astype(np.int64)
    logits[rows, tgt] += deltas[core_of, rows]
    return logits


# revision 47
# speedup vs baseline: 1.2128x; 1.2128x over previous
"""ArcFace logits on 8 Trainium2 NeuronCores — class-parallel (partial-FC) sharding.

Math: logits = SCALE * cos(arccos(clip(f_n @ w_n.T)) + MARGIN*onehot(targets))
Since cos(arccos(x)) == x, only the 1024 target entries need the margin
correction cos(t+m) = cos(m)*x - sin(m)*sqrt(1-x^2); everything else is just
the normalized matmul scaled by SCALE.

Device (SPMD, identical graph on all 8 cores, class-sharded):
  - f row-normalize (*SCALE folded in), cast bf16, PE-transpose -> fT
  - main matmul out[c,b] = wT.T @ fT in bf16 (fp32 PSUM), w-norm scaling fused
    into the PSUM->SBUF evacuation (per-partition tensor_scalar), fp16 out
  - w column norms: ACT Square -> k-presum (GPSIMD) -> one N=1 bf16
    ones-matmul per 128-class block -> DVE reciprocal + ACT Sqrt
  - margin deltas for all 1024 rows from gathered target weight rows
    (priority-demoted so the static scheduler slots it into mid-kernel slack)
DMA: all inputs + outputs on the two HWDGE rings (sync/scalar); weight chunks
prefetched two ahead; preamble loads (f, w0, w1) split across both rings.
Host: shard/transpose/concat + apply the device-computed deltas at the 1024
target positions (pure indexing; all arithmetic happens on device).
"""

import math
import os

import numpy as np

IN_F = 512
OUT_C = 100000
B = 1024
MARGIN = 0.5
SCALE = 20.0

NCORES = 8
CSH = 12800            # classes per core after padding
CPAD = CSH * NCORES    # 102400
P = 128
KT = IN_F // P         # 4 contraction subtiles
BT = B // P            # 8 batch tiles
NF = 512               # matmul moving free dim (one PSUM bank of fp32)
NB = B // NF           # 2
CW = 1280              # class chunk width streamed from DRAM
CBK = CW // P          # 10 class blocks per chunk
CHUNKS = CSH // CW     # 10
OG = 5                 # c-blocks batched per output DMA

_GRAPH = None
LAST_EXEC_TIME_NS = None


def _build_graph():
    from contextlib import ExitStack

    import concourse.bass as bass  # noqa: F401
    import concourse.tile as tile
    from concourse import bacc, mybir

    dt = mybir.dt
    AF = mybir.ActivationFunctionType
    ALU = mybir.AluOpType
    cosm = math.cos(MARGIN)
    sinm = math.sin(MARGIN)

    nc = bacc.Bacc()
    wT_e = nc.declare_dram_parameter("wT", [IN_F, CSH], dt.bfloat16, isOutput=False)
    fT_e = nc.declare_dram_parameter("fT", [IN_F, B], dt.bfloat16, isOutput=False)
    f_e = nc.declare_dram_parameter("f", [B, IN_F], dt.bfloat16, isOutput=False)
    wtg_e = nc.declare_dram_parameter("wtgt", [B, IN_F], dt.bfloat16, isOutput=False)
    out_e = nc.declare_dram_parameter("out", [CSH, B], dt.float16, isOutput=True)
    dlt_e = nc.declare_dram_parameter("delta", [P, BT], dt.float32, isOutput=True)

    wT_v = wT_e[:].rearrange("(k p) c -> p k c", p=P)   # d = k*128 + p
    fT_v = fT_e[:].rearrange("(k p) b -> p k b", p=P)   # d = k*128 + p
    f_v = f_e[:].rearrange("(t p) d -> p t d", p=P)     # b = t*128 + p
    wtg_v = wtg_e[:].rearrange("(t p) d -> p t d", p=P)

    with ExitStack() as ctx:
        tc = ctx.enter_context(tile.TileContext(nc))
        cpool = ctx.enter_context(tc.tile_pool(name="cpool", bufs=1))
        fpool = ctx.enter_context(tc.tile_pool(name="fpool", bufs=1))
        wpool = ctx.enter_context(tc.tile_pool(name="wpool", bufs=3))
        sqpool = ctx.enter_context(tc.tile_pool(name="sqpool", bufs=2))
        opool = ctx.enter_context(tc.tile_pool(name="opool", bufs=3))
        smal = ctx.enter_context(tc.tile_pool(name="smal", bufs=2))
        pn_pool = ctx.enter_context(tc.tile_pool(name="pn", bufs=1, space="PSUM"))
        po_pool = ctx.enter_context(tc.tile_pool(name="po", bufs=7, space="PSUM"))

        # ---------------- input DMAs first; fT / w0 / w1 split across both
        # HWDGE rings so the preamble pipeline fills as fast as possible ----
        fT_raw = fpool.tile([P, KT, B], dt.bfloat16)
        nc.sync.dma_start(fT_raw[:, 0 : KT // 2], fT_v[:, 0 : KT // 2])
        nc.scalar.dma_start(fT_raw[:, KT // 2 :], fT_v[:, KT // 2 :])

        def emit_load(ci, split=False):
            w_sb = wpool.tile([P, KT, CW], dt.bfloat16, tag="wchunk", name="w_sb")
            src = wT_v[:, :, ci * CW : (ci + 1) * CW]
            if split:
                nc.sync.dma_start(w_sb[:, 0 : KT // 2], src[:, 0 : KT // 2])
                nc.scalar.dma_start(w_sb[:, KT // 2 :], src[:, KT // 2 :])
            else:
                eng = nc.sync if ci % 2 == 0 else nc.scalar
                eng.dma_start(w_sb[:], src)
            return w_sb

        w_sbs = {0: emit_load(0, split=True), 1: emit_load(1, split=True)}

        # margin-path inputs (needed only mid-kernel) queue behind the above
        wt_sb = fpool.tile([P, BT, IN_F], dt.bfloat16, name="wt_sb")
        nc.sync.dma_start(wt_sb[:], wtg_v)
        f_sb = fpool.tile([P, BT, IN_F], dt.bfloat16)
        nc.scalar.dma_start(f_sb[:], f_v)

        ones_b = cpool.tile([P, 1], dt.bfloat16)
        nc.gpsimd.memset(ones_b[:], 1.0)
        ones_r = cpool.tile([1, P], dt.bfloat16)
        nc.gpsimd.memset(ones_r[:], 1.0)
        # dummy Sqrt so the one-time ACT_TABLE_LOAD (~1.3us) happens during
        # the input DMA wait instead of on the rnf critical chain
        warm = cpool.tile([1, 1], dt.float32)
        nc.scalar.activation(warm[:], ones_b[0:1, 0:1], AF.Sqrt, scale=1.0)

        # ---------------- f path: batch norms without any transposes -------
        # |f_b|^2 via Square + k-presum + ones-stationary row-matmul, then
        # SCALE*rsqrt broadcast to all partitions with a K=1 outer-product
        # matmul; fT is normalized in place of the old mul+transpose chain.
        fT_sq = sqpool.tile([P, KT, B], dt.bfloat16, tag="ftsq", name="ftsq")
        nc.scalar.activation(fT_sq[:], fT_raw[:], AF.Square)
        fsum = sqpool.tile([P, B], dt.bfloat16, tag="fsum", name="fsum")
        nc.vector.tensor_add(fsum[:], fT_sq[:, 0], fT_sq[:, 1])
        nc.vector.tensor_add(fsum[:], fsum[:], fT_sq[:, 2])
        nc.vector.tensor_add(fsum[:], fsum[:], fT_sq[:, 3])

        rnf_rec = smal.tile([1, B], dt.float32, tag="rnfrec")
        rnf_row = smal.tile([1, B], dt.bfloat16, tag="rnfrow")
        rnf_bc = fpool.tile([P, B], dt.bfloat16, name="rnfbc")
        psrs = []
        for h in range(NB):
            psr = po_pool.tile([P, NF], dt.float32, tag="pso", name="psr")
            nc.tensor.matmul(
                psr[0:1, :],
                lhsT=ones_b[:],
                rhs=fsum[:, h * NF : (h + 1) * NF],
                start=True,
                stop=True,
            )
            psrs.append(psr)
        for h in range(NB):
            # 1/nf2 at ~18 bits (values ~512, far from the undefined edges)
            nc.vector.reciprocal_approx_fast(
                rnf_rec[:, h * NF : (h + 1) * NF], psrs[h][0:1, :]
            )
        for h in range(NB):
            # sqrt(SCALE^2 / nf2) = SCALE * rsqrt(nf2)
            nc.scalar.activation(
                rnf_row[:, h * NF : (h + 1) * NF],
                rnf_rec[:, h * NF : (h + 1) * NF],
                AF.Sqrt,
                scale=SCALE * SCALE,
            )
        psbs = []
        for h in range(NB):
            psb = po_pool.tile([P, NF], dt.float32, tag="pso", name="psb")
            nc.tensor.matmul(
                psb[:],
                lhsT=ones_r[:],
                rhs=rnf_row[:, h * NF : (h + 1) * NF],
                start=True,
                stop=True,
            )
            psbs.append(psb)
        for h in range(NB):
            nc.vector.tensor_copy(rnf_bc[:, h * NF : (h + 1) * NF], psbs[h][:])

        fT = fpool.tile([P, KT, B], dt.bfloat16)
        for k in range(KT):
            nc.vector.tensor_mul(fT[:, k], fT_raw[:, k], rnf_bc[:])

        # ---------------- w-norm pipeline pieces --------------------------
        def emit_sqsum(ci):
            """ACT square of chunk ci, then DVE k-presum -> wsum bf16."""
            w_sb = w_sbs[ci]
            wsq = sqpool.tile([P, KT, CW], dt.bfloat16, tag="wsq", name="wsq")
            nc.scalar.activation(wsq[:], w_sb[:], AF.Square)
            wsum = sqpool.tile([P, CW], dt.bfloat16, tag="wsum", name="wsum")
            nc.vector.tensor_add(wsum[:], wsq[:, 0], wsq[:, 1])
            nc.vector.tensor_add(wsum[:], wsum[:], wsq[:, 2])
            nc.vector.tensor_add(wsum[:], wsum[:], wsq[:, 3])
            return wsum

        # chunk0's square: two contiguous k-half ACT ops (a strided column
        # slice would run 2x slower and head-of-line block the ACT queue);
        # the per-og k-presums then read column slices of it on DVE
        wsq0 = sqpool.tile([P, KT, CW], dt.bfloat16, tag="wsq0", name="wsq0")
        nc.scalar.activation(wsq0[:, 0:2], w_sbs[0][:, 0:2], AF.Square)
        nc.scalar.activation(wsq0[:, 2:4], w_sbs[0][:, 2:4], AF.Square)

        def emit_sqsum_part(c0, c1):
            wsum = sqpool.tile([P, c1 - c0], dt.bfloat16, tag="wsump", name="wsump")
            nc.vector.tensor_add(wsum[:], wsq0[:, 0, c0:c1], wsq0[:, 1, c0:c1])
            nc.vector.tensor_add(wsum[:], wsum[:], wsq0[:, 2, c0:c1])
            nc.vector.tensor_add(wsum[:], wsum[:], wsq0[:, 3, c0:c1])
            return wsum

        # chunk0's og0 norm slice (ready early so its evacuations can start
        # as soon as the first matmul banks stop)
        wsum0a = emit_sqsum_part(0, OG * P)

        def emit_normmm(ci, wsum, nblk=CBK, tag=""):
            """Per-class norm^2 via one N=1 ones-matmul per 128-class block."""
            nps = pn_pool.tile([P, nblk], dt.float32, tag=f"nps{tag}", name="nps")
            for cb in range(nblk):
                nc.tensor.matmul(
                    nps[:, cb : cb + 1],
                    lhsT=wsum[:, cb * P : (cb + 1) * P],
                    rhs=ones_b[:],
                    start=True,
                    stop=True,
                )
            recw = smal.tile([P, nblk], dt.float32, tag=f"recw{tag}", name="recw")
            nc.vector.reciprocal(recw[:], nps[:])
            rnw = smal.tile([P, nblk], dt.float32, tag=f"rnw{tag}", name="rnw")
            nc.scalar.activation(rnw[:], recw[:], AF.Sqrt, scale=1.0)
            return rnw

        nt2 = smal.tile([P, BT], dt.float32)
        nf2m = smal.tile([P, BT], dt.float32)
        drot = smal.tile([P, BT], dt.float32)

        def emit_margin_part(ts):
            for t in ts:
                sq = sqpool.tile([P, IN_F], dt.float32, tag="sqscratch")
                nc.scalar.activation(
                    sq[:], wt_sb[:, t], AF.Square, accum_out=nt2[:, t : t + 1]
                )
                sqf = sqpool.tile([P, IN_F], dt.float32, tag="sqscratch")
                nc.scalar.activation(
                    sqf[:], f_sb[:, t], AF.Square, accum_out=nf2m[:, t : t + 1]
                )
                prod = sqpool.tile([P, IN_F], dt.float32, tag="prodscratch")
                nc.vector.tensor_mul(prod[:], f_sb[:, t], wt_sb[:, t])
                nc.vector.reduce_sum(
                    drot[:, t : t + 1], prod[:], axis=mybir.AxisListType.X
                )

        def emit_margin_tail():
            rec_t = smal.tile([P, BT], dt.float32)
            nc.vector.reciprocal(rec_t[:], nt2[:])
            rnt = smal.tile([P, BT], dt.float32)
            nc.scalar.activation(rnt[:], rec_t[:], AF.Sqrt, scale=1.0)
            rec_f = smal.tile([P, BT], dt.float32)
            nc.vector.reciprocal(rec_f[:], nf2m[:])
            rnf20 = smal.tile([P, BT], dt.float32)
            nc.scalar.activation(rnf20[:], rec_f[:], AF.Sqrt, scale=SCALE * SCALE)
            u = smal.tile([P, BT], dt.float32)
            nc.vector.tensor_mul(u[:], drot[:], rnf20[:])
            nc.vector.tensor_mul(u[:], u[:], rnt[:])          # u = SCALE * cos_t
            t1 = smal.tile([P, BT], dt.float32)
            nc.vector.tensor_mul(t1[:], u[:], u[:])
            nc.vector.tensor_scalar(t1[:], t1[:], -1.0, SCALE * SCALE, ALU.mult, ALU.add)
            nc.vector.tensor_scalar_max(t1[:], t1[:], 0.0)    # max(S^2 - u^2, 0)
            s_t = smal.tile([P, BT], dt.float32)
            nc.scalar.activation(s_t[:], t1[:], AF.Sqrt, scale=1.0)  # SCALE*sin_t
            t2 = smal.tile([P, BT], dt.float32)
            nc.vector.tensor_scalar_mul(t2[:], s_t[:], -sinm)
            t3 = smal.tile([P, BT], dt.float32)
            nc.vector.tensor_scalar_mul(t3[:], u[:], cosm - 1.0)
            delta = smal.tile([P, BT], dt.float32)
            nc.vector.tensor_add(delta[:], t2[:], t3[:])
            nc.gpsimd.dma_start(dlt_e[:], delta[:])

        # ---------------- main class loop ---------------------------------
        # chunk0's norm runs in the preamble (PE waits on it once);
        # chunk i+1's square/presum is emitted at og1 of chunk i and its
        # norm-matmuls at the end of chunk i, so steady-state PE never waits.
        rnw = emit_normmm(0, wsum0a, nblk=OG)
        rnw_off = 0

        for ci in range(CHUNKS):
            w_sb = w_sbs.pop(ci)
            for og in range(CBK // OG):
                if ci == 0 and og == 1:
                    # second half of chunk0's norm, between og0 and og1 MMs
                    rnw = emit_normmm(
                        0, emit_sqsum_part(OG * P, CW), nblk=OG
                    )
                    rnw_off = OG
                if og == 0 and ci + 2 < CHUNKS:
                    w_sbs[ci + 2] = emit_load(ci + 2)
                osb = opool.tile([P, OG, NB * NF], dt.float16, tag="osb")
                for cbi in range(OG):
                    cb = og * OG + cbi
                    psos = [
                        po_pool.tile([P, NF], dt.float32, tag="pso", name=f"pso{nb}")
                        for nb in range(NB)
                    ]
                    for k in range(KT):
                        for nb in range(NB):
                            nc.tensor.matmul(
                                psos[nb][:],
                                lhsT=w_sb[:, k, cb * P : (cb + 1) * P],
                                rhs=fT[:, k, nb * NF : (nb + 1) * NF],
                                start=(k == 0),
                                stop=(k == KT - 1),
                            )
                    if og == 1 and cbi == 3 and ci + 1 < CHUNKS:
                        # norm-matmuls for the next chunk ride mid-og1 (wsum
                        # is ready by now), so rnw(ci+1) beats the chunk
                        # boundary and og0's evacuations never wait on it
                        new_rnw = emit_normmm(ci + 1, next_wsum)
                    ri = cb - rnw_off
                    for nb in range(NB):
                        if (cb + nb) % 2 == 0:
                            nc.scalar.activation(
                                osb[:, cbi, nb * NF : (nb + 1) * NF],
                                psos[nb][:], AF.Copy,
                                scale=rnw[:, ri : ri + 1],
                            )
                        else:
                            nc.vector.tensor_scalar_mul(
                                osb[:, cbi, nb * NF : (nb + 1) * NF],
                                psos[nb][:], rnw[:, ri : ri + 1],
                            )
                row0 = ci * CW + og * OG * P
                if ci == CHUNKS - 1 and og == 1:
                    # fine-grained split of the final burst across both
                    # rings, per block, so the drain tail is minimal
                    for g in range(OG):
                        eng = nc.sync if g % 2 == 0 else nc.scalar
                        eng.dma_start(
                            out_e[row0 + g * P : row0 + (g + 1) * P, :].rearrange(
                                "(g p) b -> p g b", p=P
                            ),
                            osb[:, g : g + 1],
                        )
                else:
                    dma_eng = nc.sync if og % 2 == 0 else nc.scalar
                    dma_eng.dma_start(
                        out_e[row0 : row0 + OG * P, :].rearrange(
                            "(g p) b -> p g b", p=P
                        ),
                        osb[:],
                    )
                if og == 0 and ci + 1 < CHUNKS:
                    # square + presum for the next chunk start at og0's end:
                    # ACT runs it after og0's evacuations, DVE after og0's,
                    # and wsum is ready before mid-og1
                    next_wsum = emit_sqsum(ci + 1)
            if ci + 1 < CHUNKS:
                rnw = new_rnw
                rnw_off = 0
            # margin work split into small slots across chunks so its
            # ACT/DVE bursts never starve the evacuations; tile_wait_until
            # pins each slot to mid-kernel model time so the static scheduler
            # cannot hoist it to the stream heads (it would head-of-line
            # block the f path while waiting on the wtgt/f DMAs)
            if 2 <= ci <= 5:
                with tc.tile_wait_until(0.055 + 0.02 * (ci - 2)):
                    emit_margin_part(range((ci - 2) * 2, (ci - 2) * 2 + 2))
            elif ci == 6:
                with tc.tile_wait_until(0.135):
                    emit_margin_tail()

    nc.finalize()
    return nc


def _prep_inputs(features, targets, weights):
    import ml_dtypes

    f32 = np.asarray(features, dtype=np.float32)
    fbf = np.ascontiguousarray(f32.astype(ml_dtypes.bfloat16))
    fTbf = np.ascontiguousarray(fbf.T)
    tgt = np.asarray(targets).astype(np.int64)
    w = np.asarray(weights, dtype=np.float32)

    wpad = np.zeros((CPAD, IN_F), dtype=np.float32)
    wpad[:OUT_C] = w
    wpad[OUT_C:, 0] = 1.0  # unit-norm filler rows: no inf/nan anywhere

    in_maps = []
    for i in range(NCORES):
        sh = wpad[i * CSH : (i + 1) * CSH]
        wT = np.ascontiguousarray(sh.astype(ml_dtypes.bfloat16).T)
        loc = np.clip(tgt - i * CSH, 0, CSH - 1)
        wtgt = np.ascontiguousarray(sh[loc].astype(ml_dtypes.bfloat16))
        in_maps.append({"wT": wT, "fT": fTbf, "f": fbf, "wtgt": wtgt})
    return in_maps, tgt


def kernel(features, targets, weights):
    global _GRAPH, LAST_EXEC_TIME_NS
    from concourse.bass_utils import run_bass_kernel_spmd

    if _GRAPH is None:
        _GRAPH = _build_graph()
    nc = _GRAPH

    in_maps, tgt = _prep_inputs(features, targets, weights)

    trace = bool(int(os.environ.get("BASS_KERNEL_TRACE", "0")))
    res = run_bass_kernel_spmd(nc, in_maps, core_ids=list(range(NCORES)), trace=trace)
    LAST_EXEC_TIME_NS = res.exec_time_ns

    outs = [res.results[i]["out"] for i in range(NCORES)]       # [CSH, B] f16
    full = np.concatenate(outs, axis=0)[:OUT_C]                 # [OUT_C, B]
    logits = np.ascontiguousarray(full.T, dtype=np.float32)     # [B, OUT_C]

    # apply device-computed margin deltas at the 1024 target positions
    deltas = np.stack(
        [res.results[i]["delta"].T.reshape(B) for i in range(NCORES)]
    )  # [NCORES, B]; delta[p, t] -> b = t*128 + p
    rows = np.arange(B)
    core_of = (tgt // CSH).astype(np.int64)
    logits[rows, tgt] += deltas[core_of, rows]
    return logits


# revision 48
# speedup vs baseline: 1.2130x; 1.0002x over previous
"""ArcFace logits on 8 Trainium2 NeuronCores — class-parallel (partial-FC) sharding.

Math: logits = SCALE * cos(arccos(clip(f_n @ w_n.T)) + MARGIN*onehot(targets))
Since cos(arccos(x)) == x, only the 1024 target entries need the margin
correction cos(t+m) = cos(m)*x - sin(m)*sqrt(1-x^2); everything else is just
the normalized matmul scaled by SCALE.

Device (SPMD, identical graph on all 8 cores, class-sharded):
  - f row-normalize (*SCALE folded in), cast bf16, PE-transpose -> fT
  - main matmul out[c,b] = wT.T @ fT in bf16 (fp32 PSUM), w-norm scaling fused
    into the PSUM->SBUF evacuation (per-partition tensor_scalar), fp16 out
  - w column norms: ACT Square -> k-presum (GPSIMD) -> one N=1 bf16
    ones-matmul per 128-class block -> DVE reciprocal + ACT Sqrt
  - margin deltas for all 1024 rows from gathered target weight rows
    (priority-demoted so the static scheduler slots it into mid-kernel slack)
DMA: all inputs + outputs on the two HWDGE rings (sync/scalar); weight chunks
prefetched two ahead; preamble loads (f, w0, w1) split across both rings.
Host: shard/transpose/concat + apply the device-computed deltas at the 1024
target positions (pure indexing; all arithmetic happens on device).
"""

import math
import os

import numpy as np

IN_F = 512
OUT_C = 100000
B = 1024
MARGIN = 0.5
SCALE = 20.0

NCORES = 8
CSH = 12800            # classes per core after padding
CPAD = CSH * NCORES    # 102400
P = 128
KT = IN_F // P         # 4 contraction subtiles
BT = B // P            # 8 batch tiles
NF = 512               # matmul moving free dim (one PSUM bank of fp32)
NB = B // NF           # 2
CW = 1280              # class chunk width streamed from DRAM
CBK = CW // P          # 10 class blocks per chunk
CHUNKS = CSH // CW     # 10
OG = 5                 # c-blocks batched per output DMA

_GRAPH = None
LAST_EXEC_TIME_NS = None


def _build_graph():
    from contextlib import ExitStack

    import concourse.bass as bass  # noqa: F401
    import concourse.tile as tile
    from concourse import bacc, mybir

    dt = mybir.dt
    AF = mybir.ActivationFunctionType
    ALU = mybir.AluOpType
    cosm = math.cos(MARGIN)
    sinm = math.sin(MARGIN)

    nc = bacc.Bacc()
    wT_e = nc.declare_dram_parameter("wT", [IN_F, CSH], dt.bfloat16, isOutput=False)
    fT_e = nc.declare_dram_parameter("fT", [IN_F, B], dt.bfloat16, isOutput=False)
    f_e = nc.declare_dram_parameter("f", [B, IN_F], dt.bfloat16, isOutput=False)
    wtg_e = nc.declare_dram_parameter("wtgt", [B, IN_F], dt.bfloat16, isOutput=False)
    out_e = nc.declare_dram_parameter("out", [CSH, B], dt.float16, isOutput=True)
    dlt_e = nc.declare_dram_parameter("delta", [P, BT], dt.float32, isOutput=True)

    wT_v = wT_e[:].rearrange("(k p) c -> p k c", p=P)   # d = k*128 + p
    fT_v = fT_e[:].rearrange("(k p) b -> p k b", p=P)   # d = k*128 + p
    f_v = f_e[:].rearrange("(t p) d -> p t d", p=P)     # b = t*128 + p
    wtg_v = wtg_e[:].rearrange("(t p) d -> p t d", p=P)

    with ExitStack() as ctx:
        tc = ctx.enter_context(tile.TileContext(nc))
        cpool = ctx.enter_context(tc.tile_pool(name="cpool", bufs=1))
        fpool = ctx.enter_context(tc.tile_pool(name="fpool", bufs=1))
        wpool = ctx.enter_context(tc.tile_pool(name="wpool", bufs=3))
        sqpool = ctx.enter_context(tc.tile_pool(name="sqpool", bufs=2))
        opool = ctx.enter_context(tc.tile_pool(name="opool", bufs=3))
        smal = ctx.enter_context(tc.tile_pool(name="smal", bufs=2))
        pn_pool = ctx.enter_context(tc.tile_pool(name="pn", bufs=1, space="PSUM"))
        po_pool = ctx.enter_context(tc.tile_pool(name="po", bufs=7, space="PSUM"))

        # ---------------- input DMAs first; fT / w0 / w1 split across both
        # HWDGE rings so the preamble pipeline fills as fast as possible ----
        fT_raw = fpool.tile([P, KT, B], dt.bfloat16)
        nc.sync.dma_start(fT_raw[:, 0 : KT // 2], fT_v[:, 0 : KT // 2])
        nc.scalar.dma_start(fT_raw[:, KT // 2 :], fT_v[:, KT // 2 :])

        def emit_load(ci, split=False):
            w_sb = wpool.tile([P, KT, CW], dt.bfloat16, tag="wchunk", name="w_sb")
            src = wT_v[:, :, ci * CW : (ci + 1) * CW]
            if split:
                nc.sync.dma_start(w_sb[:, 0 : KT // 2], src[:, 0 : KT // 2])
                nc.scalar.dma_start(w_sb[:, KT // 2 :], src[:, KT // 2 :])
            else:
                eng = nc.sync if ci % 2 == 0 else nc.scalar
                eng.dma_start(w_sb[:], src)
            return w_sb

        w_sbs = {0: emit_load(0, split=True), 1: emit_load(1, split=True)}

        # margin-path inputs (needed only mid-kernel) queue behind the above
        wt_sb = fpool.tile([P, BT, IN_F], dt.bfloat16, name="wt_sb")
        nc.sync.dma_start(wt_sb[:], wtg_v)
        f_sb = fpool.tile([P, BT, IN_F], dt.bfloat16)
        nc.scalar.dma_start(f_sb[:], f_v)

        ones_b = cpool.tile([P, 1], dt.bfloat16)
        nc.gpsimd.memset(ones_b[:], 1.0)
        ones_r = cpool.tile([1, P], dt.bfloat16)
        nc.gpsimd.memset(ones_r[:], 1.0)
        # dummy Sqrt so the one-time ACT_TABLE_LOAD (~1.3us) happens during
        # the input DMA wait instead of on the rnf critical chain
        warm = cpool.tile([1, 1], dt.float32)
        nc.scalar.activation(warm[:], ones_b[0:1, 0:1], AF.Sqrt, scale=1.0)

        # ---------------- f path: batch norms without any transposes -------
        # |f_b|^2 via Square + k-presum + ones-stationary row-matmul, then
        # SCALE*rsqrt broadcast to all partitions with a K=1 outer-product
        # matmul; fT is normalized in place of the old mul+transpose chain.
        fT_sq = sqpool.tile([P, KT, B], dt.bfloat16, tag="ftsq", name="ftsq")
        nc.scalar.activation(fT_sq[:], fT_raw[:], AF.Square)
        fsum = sqpool.tile([P, B], dt.bfloat16, tag="fsum", name="fsum")
        nc.vector.tensor_add(fsum[:], fT_sq[:, 0], fT_sq[:, 1])
        nc.vector.tensor_add(fsum[:], fsum[:], fT_sq[:, 2])
        nc.vector.tensor_add(fsum[:], fsum[:], fT_sq[:, 3])

        rnf_rec = smal.tile([1, B], dt.float32, tag="rnfrec")
        rnf_row = smal.tile([1, B], dt.bfloat16, tag="rnfrow")
        rnf_bc = fpool.tile([P, B], dt.bfloat16, name="rnfbc")
        psrs = []
        for h in range(NB):
            psr = po_pool.tile([P, NF], dt.float32, tag="pso", name="psr")
            nc.tensor.matmul(
                psr[0:1, :],
                lhsT=ones_b[:],
                rhs=fsum[:, h * NF : (h + 1) * NF],
                start=True,
                stop=True,
            )
            psrs.append(psr)
        for h in range(NB):
            # 1/nf2 at ~18 bits (values ~512, far from the undefined edges)
            nc.vector.reciprocal_approx_fast(
                rnf_rec[:, h * NF : (h + 1) * NF], psrs[h][0:1, :]
            )
        for h in range(NB):
            # sqrt(SCALE^2 / nf2) = SCALE * rsqrt(nf2)
            nc.scalar.activation(
                rnf_row[:, h * NF : (h + 1) * NF],
                rnf_rec[:, h * NF : (h + 1) * NF],
                AF.Sqrt,
                scale=SCALE * SCALE,
            )
        psbs = []
        for h in range(NB):
            psb = po_pool.tile([P, NF], dt.float32, tag="pso", name="psb")
            nc.tensor.matmul(
                psb[:],
                lhsT=ones_r[:],
                rhs=rnf_row[:, h * NF : (h + 1) * NF],
                start=True,
                stop=True,
            )
            psbs.append(psb)
        for h in range(NB):
            nc.vector.tensor_copy(rnf_bc[:, h * NF : (h + 1) * NF], psbs[h][:])

        fT = fpool.tile([P, KT, B], dt.bfloat16)
        for k in range(KT):
            nc.vector.tensor_mul(fT[:, k], fT_raw[:, k], rnf_bc[:])

        # ---------------- w-norm pipeline pieces --------------------------
        def emit_sqsum(ci):
            """ACT square of chunk ci (two contiguous k-half ops so neither
            blocks the ACT queue long), then DVE k-presum -> wsum bf16."""
            w_sb = w_sbs[ci]
            wsq = sqpool.tile([P, KT, CW], dt.bfloat16, tag="wsq", name="wsq")
            nc.scalar.activation(wsq[:, 0:2], w_sb[:, 0:2], AF.Square)
            nc.scalar.activation(wsq[:, 2:4], w_sb[:, 2:4], AF.Square)
            wsum = sqpool.tile([P, CW], dt.bfloat16, tag="wsum", name="wsum")
            nc.vector.tensor_add(wsum[:], wsq[:, 0], wsq[:, 1])
            nc.vector.tensor_add(wsum[:], wsum[:], wsq[:, 2])
            nc.vector.tensor_add(wsum[:], wsum[:], wsq[:, 3])
            return wsum

        # chunk0's square: two contiguous k-half ACT ops (a strided column
        # slice would run 2x slower and head-of-line block the ACT queue);
        # the per-og k-presums then read column slices of it on DVE
        wsq0 = sqpool.tile([P, KT, CW], dt.bfloat16, tag="wsq0", name="wsq0")
        nc.scalar.activation(wsq0[:, 0:2], w_sbs[0][:, 0:2], AF.Square)
        nc.scalar.activation(wsq0[:, 2:4], w_sbs[0][:, 2:4], AF.Square)

        def emit_sqsum_part(c0, c1):
            wsum = sqpool.tile([P, c1 - c0], dt.bfloat16, tag="wsump", name="wsump")
            nc.vector.tensor_add(wsum[:], wsq0[:, 0, c0:c1], wsq0[:, 1, c0:c1])
            nc.vector.tensor_add(wsum[:], wsum[:], wsq0[:, 2, c0:c1])
            nc.vector.tensor_add(wsum[:], wsum[:], wsq0[:, 3, c0:c1])
            return wsum

        # chunk0's og0 norm slice (ready early so its evacuations can start
        # as soon as the first matmul banks stop)
        wsum0a = emit_sqsum_part(0, OG * P)

        def emit_normmm(ci, wsum, nblk=CBK, tag=""):
            """Per-class norm^2 via one N=1 ones-matmul per 128-class block."""
            nps = pn_pool.tile([P, nblk], dt.float32, tag=f"nps{tag}", name="nps")
            for cb in range(nblk):
                nc.tensor.matmul(
                    nps[:, cb : cb + 1],
                    lhsT=wsum[:, cb * P : (cb + 1) * P],
                    rhs=ones_b[:],
                    start=True,
                    stop=True,
                )
            recw = smal.tile([P, nblk], dt.float32, tag=f"recw{tag}", name="recw")
            nc.vector.reciprocal(recw[:], nps[:])
            rnw = smal.tile([P, nblk], dt.float32, tag=f"rnw{tag}", name="rnw")
            nc.scalar.activation(rnw[:], recw[:], AF.Sqrt, scale=1.0)
            return rnw

        nt2 = smal.tile([P, BT], dt.float32)
        nf2m = smal.tile([P, BT], dt.float32)
        drot = smal.tile([P, BT], dt.float32)

        def emit_margin_part(ts):
            for t in ts:
                sq = sqpool.tile([P, IN_F], dt.float32, tag="sqscratch")
                nc.scalar.activation(
                    sq[:], wt_sb[:, t], AF.Square, accum_out=nt2[:, t : t + 1]
                )
                sqf = sqpool.tile([P, IN_F], dt.float32, tag="sqscratch")
                nc.scalar.activation(
                    sqf[:], f_sb[:, t], AF.Square, accum_out=nf2m[:, t : t + 1]
                )
                prod = sqpool.tile([P, IN_F], dt.float32, tag="prodscratch")
                nc.vector.tensor_mul(prod[:], f_sb[:, t], wt_sb[:, t])
                nc.vector.reduce_sum(
                    drot[:, t : t + 1], prod[:], axis=mybir.AxisListType.X
                )

        def emit_margin_tail():
            rec_t = smal.tile([P, BT], dt.float32)
            nc.vector.reciprocal(rec_t[:], nt2[:])
            rnt = smal.tile([P, BT], dt.float32)
            nc.scalar.activation(rnt[:], rec_t[:], AF.Sqrt, scale=1.0)
            rec_f = smal.tile([P, BT], dt.float32)
            nc.vector.reciprocal(rec_f[:], nf2m[:])
            rnf20 = smal.tile([P, BT], dt.float32)
            nc.scalar.activation(rnf20[:], rec_f[:], AF.Sqrt, scale=SCALE * SCALE)
            u = smal.tile([P, BT], dt.float32)
            nc.vector.tensor_mul(u[:], drot[:], rnf20[:])
            nc.vector.tensor_mul(u[:], u[:], rnt[:])          # u = SCALE * cos_t
            t1 = smal.tile([P, BT], dt.float32)
            nc.vector.tensor_mul(t1[:], u[:], u[:])
            nc.vector.tensor_scalar(t1[:], t1[:], -1.0, SCALE * SCALE, ALU.mult, ALU.add)
            nc.vector.tensor_scalar_max(t1[:], t1[:], 0.0)    # max(S^2 - u^2, 0)
            s_t = smal.tile([P, BT], dt.float32)
            nc.scalar.activation(s_t[:], t1[:], AF.Sqrt, scale=1.0)  # SCALE*sin_t
            t2 = smal.tile([P, BT], dt.float32)
            nc.vector.tensor_scalar_mul(t2[:], s_t[:], -sinm)
            t3 = smal.tile([P, BT], dt.float32)
            nc.vector.tensor_scalar_mul(t3[:], u[:], cosm - 1.0)
            delta = smal.tile([P, BT], dt.float32)
            nc.vector.tensor_add(delta[:], t2[:], t3[:])
            nc.gpsimd.dma_start(dlt_e[:], delta[:])

        # ---------------- main class loop ---------------------------------
        # chunk0's norm runs in the preamble (PE waits on it once);
        # chunk i+1's square/presum is emitted at og1 of chunk i and its
        # norm-matmuls at the end of chunk i, so steady-state PE never waits.
        rnw = emit_normmm(0, wsum0a, nblk=OG)
        rnw_off = 0

        for ci in range(CHUNKS):
            w_sb = w_sbs.pop(ci)
            for og in range(CBK // OG):
                if ci == 0 and og == 1:
                    # second half of chunk0's norm, between og0 and og1 MMs
                    rnw = emit_normmm(
                        0, emit_sqsum_part(OG * P, CW), nblk=OG
                    )
                    rnw_off = OG
                if og == 0 and ci + 2 < CHUNKS:
                    w_sbs[ci + 2] = emit_load(ci + 2)
                osb = opool.tile([P, OG, NB * NF], dt.float16, tag="osb")
                for cbi in range(OG):
                    cb = og * OG + cbi
                    psos = [
                        po_pool.tile([P, NF], dt.float32, tag="pso", name=f"pso{nb}")
                        for nb in range(NB)
                    ]
                    for k in range(KT):
                        for nb in range(NB):
                            nc.tensor.matmul(
                                psos[nb][:],
                                lhsT=w_sb[:, k, cb * P : (cb + 1) * P],
                                rhs=fT[:, k, nb * NF : (nb + 1) * NF],
                                start=(k == 0),
                                stop=(k == KT - 1),
                            )
                    if og == 1 and cbi == 3 and ci + 1 < CHUNKS:
                        # norm-matmuls for the next chunk ride mid-og1 (wsum
                        # is ready by now), so rnw(ci+1) beats the chunk
                        # boundary and og0's evacuations never wait on it
                        new_rnw = emit_normmm(ci + 1, next_wsum)
                    ri = cb - rnw_off
                    for nb in range(NB):
                        if (cb + nb) % 2 == 0:
                            nc.scalar.activation(
                                osb[:, cbi, nb * NF : (nb + 1) * NF],
                                psos[nb][:], AF.Copy,
                                scale=rnw[:, ri : ri + 1],
                            )
                        else:
                            nc.vector.tensor_scalar_mul(
                                osb[:, cbi, nb * NF : (nb + 1) * NF],
                                psos[nb][:], rnw[:, ri : ri + 1],
                            )
                row0 = ci * CW + og * OG * P
                if ci == CHUNKS - 1 and og == 1:
                    # fine-grained split of the final burst across both
                    # rings, per block, so the drain tail is minimal
                    for g in range(OG):
                        eng = nc.sync if g % 2 == 0 else nc.scalar
                        eng.dma_start(
                            out_e[row0 + g * P : row0 + (g + 1) * P, :].rearrange(
                                "(g p) b -> p g b", p=P
                            ),
                            osb[:, g : g + 1],
                        )
                else:
                    dma_eng = nc.sync if og % 2 == 0 else nc.scalar
                    dma_eng.dma_start(
                        out_e[row0 : row0 + OG * P, :].rearrange(
                            "(g p) b -> p g b", p=P
                        ),
                        osb[:],
                    )
                if og == 0 and ci + 1 < CHUNKS:
                    # square + presum for the next chunk start at og0's end:
                    # ACT runs it after og0's evacuations, DVE after og0's,
                    # and wsum is ready before mid-og1
                    next_wsum = emit_sqsum(ci + 1)
            if ci + 1 < CHUNKS:
                rnw = new_rnw
                rnw_off = 0
            # margin work split into small slots across chunks so its
            # ACT/DVE bursts never starve the evacuations; tile_wait_until
            # pins each slot to mid-kernel model time so the static scheduler
            # cannot hoist it to the stream heads (it would head-of-line
            # block the f path while waiting on the wtgt/f DMAs)
            if 2 <= ci <= 5:
                with tc.tile_wait_until(0.055 + 0.02 * (ci - 2)):
                    emit_margin_part(range((ci - 2) * 2, (ci - 2) * 2 + 2))
            elif ci == 6:
                with tc.tile_wait_until(0.135):
                    emit_margin_tail()

    nc.finalize()
    return nc


def _prep_inputs(features, targets, weights):
    import ml_dtypes

    f32 = np.asarray(features, dtype=np.float32)
    fbf = np.ascontiguousarray(f32.astype(ml_dtypes.bfloat16))
    fTbf = np.ascontiguousarray(fbf.T)
    tgt = np.asarray(targets).astype(np.int64)
    w = np.asarray(weights, dtype=np.float32)

    wpad = np.zeros((CPAD, IN_F), dtype=np.float32)
    wpad[:OUT_C] = w
    wpad[OUT_C:, 0] = 1.0  # unit-norm filler rows: no inf/nan anywhere

    in_maps = []
    for i in range(NCORES):
        sh = wpad[i * CSH : (i + 1) * CSH]
        wT = np.ascontiguousarray(sh.astype(ml_dtypes.bfloat16).T)
        loc = np.clip(tgt - i * CSH, 0, CSH - 1)
        wtgt = np.ascontiguousarray(sh[loc].astype(ml_dtypes.bfloat16))
        in_maps.append({"wT": wT, "fT": fTbf, "f": fbf, "wtgt": wtgt})
    return in_maps, tgt


def kernel(features, targets, weights):
    global _GRAPH, LAST_EXEC_TIME_NS
    from concourse.bass_utils import run_bass_kernel_spmd

    if _GRAPH is None:
        _GRAPH = _build_graph()
    nc = _GRAPH

    in_maps, tgt = _prep_inputs(features, targets, weights)

    trace = bool(int(os.environ.get("BASS_KERNEL_TRACE", "0")))
    res = run_bass_kernel_spmd(nc, in_maps, core_ids=list(range(NCORES)), trace=trace)
    LAST_EXEC_TIME_NS = res.exec_time_ns

    outs = [res.results[i]["out"] for i in range(NCORES)]       # [CSH, B] f16
    full = np.concatenate(outs, axis=0)[:OUT_C]                 # [OUT_C, B]
    logits = np.ascontiguousarray(full.T, dtype=np.float32)     # [B, OUT_C]

    # apply device-computed margin deltas at the 1024 target positions
    deltas = np.stack(
        [res.results[i]["delta"].T.reshape(B) for i in range(NCORES)]
    )  # [NCORES, B]; delta[p, t] -> b = t*128 + p
    rows = np.arange(B)
    core_of = (tgt // CSH).astype(np.int64)
    logits[rows, tgt] += deltas[core_of, rows]
    return logits
